# revision 55
# baseline (speedup 1.0000x reference)
"""Trainium2 Bass kernel for nn_EssentialMatixModule.

Dual-softmax cross-attention (LoFTR-style) + bilinear feature + projection.
Data-parallel over batch across 8 cores; proj output-sharded with chunked
AllGathers of the (bf16) feature matrix overlapping the attention phase.

Structure: LN stats (x split over 3 DMA queues), then one fused
software-pipelined loop: per-item QKV streams into the attention blocks
(item-major order, mh-major tail), each block is a 3-stage pipeline
(scores+exp -> colsums+up/us -> f), with E^2 / 1/Zr / vpl emitted
per-row-tile right after each exp so every PE matmul's dependencies are
at least one block old.  Column sums stream through the PE with a
stationary ones vector; 1/sigma is broadcast by DMA (f32 exact).
Feature chunks AllGather in half-batches (permuted batch order, undone
on the host) and the output projection is prefetched and interleaved so
the serial tail is short.  The scalar queue carries only activations
during attention.
"""

import sys

sys.path.insert(0, "/opt/trn_rl_repo")

from contextlib import ExitStack

import ml_dtypes
import numpy as np

import concourse.bass as bass
import concourse.tile as tile
from concourse import bacc, mybir
from concourse.bass_utils import run_bass_kernel_spmd

B, C, HG, WG = 64, 256, 24, 24
N = HG * WG  # 576
H, HD = 3, 64
F = H * HD  # 192
SCALE = HD**-0.5
EPS = 1e-5
NCORES = 8
BP = B // NCORES  # 8 items per core
NT = [128, 128, 128, 128, 64]  # token tiles (sum=576)
# free-dim chunks for N=576 psum; 64-chunk first so each matmul pair ends
# with a 512-col stream that hides the next LDWEIGHTS
NCH = [(512, 64), (0, 512)]
DE = 70  # hd + 6 pos dims
PADMH = 4992  # 39*128, per-(map,head) padded feat block
DIMS = 6 * PADMH  # 29952
OS = 512 // NCORES  # 64 output cols per core
F32 = mybir.dt.float32
BF16 = mybir.dt.bfloat16
AX = mybir.ActivationFunctionType
OP = mybir.AluOpType

# half-batch gather row order: B' = [items 0-3 of each core, items 4-7 of each core]
BORDER = np.array(
    [8 * c + i for i in (0,) for c in range(0)]  # placeholder, built below
)
_rows = []
for c in range(NCORES):
    for i in range(4):
        _rows.append(8 * c + i)
for c in range(NCORES):
    for i in range(4, 8):
        _rows.append(8 * c + i)
BORDER = np.array(_rows)  # BORDER[r] = original batch index of permuted row r


def _host_prep(ln_w, ln_b, qkv_w, proj_w, proj_b):
    ln_w = ln_w.astype(np.float64)
    ln_b = ln_b.astype(np.float64)
    qw = qkv_w.astype(np.float64)
    Wp = qw * ln_w[None, :]  # [576, C]
    r = Wp.sum(axis=1)  # [576]
    t = qw @ ln_b  # [576]

    # per-side packing: side0 tiles hold [k_h; q_h], side1 [q_h; k_h] so the
    # attention matmul operands always share a partition base
    def col(fsl, scale):
        return np.concatenate([Wp[fsl] * scale, (r[fsl] * scale)[:, None],
                               (t[fsl] * scale)[:, None]], axis=1).T

    wqk = np.zeros((2, C + 2, 3 * 128), np.float32)
    for h in range(H):
        qr = slice(h * HD, (h + 1) * HD)
        kr = slice(F + h * HD, F + (h + 1) * HD)
        qcols = col(qr, SCALE)  # [C+2, 64]
        kcols = col(kr, 1.0)
        wqk[0, :, h * 128 : h * 128 + 64] = kcols
        wqk[0, :, h * 128 + 64 : h * 128 + 128] = qcols
        wqk[1, :, h * 128 : h * 128 + 64] = qcols
        wqk[1, :, h * 128 + 64 : h * 128 + 128] = kcols
    wqk = wqk.astype(ml_dtypes.bfloat16)

    wv = np.zeros((C + 2, F), np.float32)
    wv[:C] = Wp[2 * F :].T
    wv[C] = r[2 * F :]
    wv[C + 1] = t[2 * F :]
    wv = wv.astype(ml_dtypes.bfloat16)

    ys = np.linspace(-1.0, 1.0, HG)
    xs = np.linspace(-1.0, 1.0, WG)
    p3 = np.tile(ys, WG)
    p4 = np.repeat(xs, HG)
    pos = np.stack([p3 * p3, p4 * p4, p3 * p4, p3, p4, np.ones_like(p3)], axis=1)
    pos_pad = np.zeros((640, 6), np.float32)
    pos_pad[:N] = pos

    pwt = np.zeros((DIMS, 512), np.float32)
    for mh in range(6):
        blk = proj_w[:, mh * 4900 : (mh + 1) * 4900]  # [512, 4900]
        pwt[mh * PADMH : mh * PADMH + 4900] = blk.T
    pwt = pwt.astype(ml_dtypes.bfloat16)
    return wqk, wv, pos_pad, pwt


def _build():
    nc = bacc.Bacc()
    x1d = nc.declare_dram_parameter("x1s", [BP, C, N], BF16, isOutput=False)
    x2d = nc.declare_dram_parameter("x2s", [BP, C, N], BF16, isOutput=False)
    wqkd = nc.declare_dram_parameter("wqk", [2, C + 2, 3 * 128], BF16, isOutput=False)
    wvd = nc.declare_dram_parameter("wv", [C + 2, F], BF16, isOutput=False)
    posd = nc.declare_dram_parameter("pos", [640, 6], F32, isOutput=False)
    pwtd = nc.declare_dram_parameter("pwt", [DIMS, OS], BF16, isOutput=False)
    pbd = nc.declare_dram_parameter("pb", [1, OS], F32, isOutput=False)
    outd = nc.declare_dram_parameter("out", [B, OS], F32, isOutput=True)
    statsd = nc.dram_tensor("statsd", [2, 2, BP, N], BF16)  # (negmu, sigma)
    isvd = nc.dram_tensor("isvd", [2, BP, N], F32)  # 1/sigma rows, f32
    feat8d = [nc.dram_tensor(f"feat8_{j}", [BP, PADMH], BF16) for j in range(6)]
    # two half-batch gather outputs per mh chunk (items 0-3, items 4-7)
    featAG = [
        [
            nc.dram_tensor(f"featAG_{j}_{hf}", [B // 2, PADMH], BF16, addr_space="Shared")
            for hf in range(2)
        ]
        for j in range(6)
    ]
    xd = [x1d, x2d]

    def bcast_p(sl, p):
        return bass.AP(tensor=sl.tensor, offset=sl.offset, ap=[[0, p]] + list(sl.ap))

    with ExitStack() as ctx:
        tc = ctx.enter_context(tile.TileContext(nc))
        const = ctx.enter_context(tc.tile_pool(name="const", bufs=1))
        xres = ctx.enter_context(tc.tile_pool(name="xres", bufs=1))
        stats = ctx.enter_context(tc.tile_pool(name="stats", bufs=1))
        tmp = ctx.enter_context(tc.tile_pool(name="tmp", bufs=2))
        sb_qk = ctx.enter_context(tc.tile_pool(name="sbqk", bufs=1))
        sb_vp = ctx.enter_context(tc.tile_pool(name="sbvp", bufs=1))
        epool = ctx.enter_context(tc.tile_pool(name="epool", bufs=11))
        e2pool = ctx.enter_context(tc.tile_pool(name="e2pool", bufs=11))
        zpool = ctx.enter_context(tc.tile_pool(name="zpool", bufs=3))
        upool = ctx.enter_context(tc.tile_pool(name="upool", bufs=8))
        vlpool = ctx.enter_context(tc.tile_pool(name="vlpool", bufs=12))
        fpool = ctx.enter_context(tc.tile_pool(name="fpool", bufs=3))
        ftpool = ctx.enter_context(tc.tile_pool(name="ftpool", bufs=3))
        opool = ctx.enter_context(tc.tile_pool(name="opool", bufs=2))
        # PSUM: tag pA [128,576]x2 = 4 banks; pZ [128,576]x1 = 2; pU [128,192]x2 = 2
        psA = ctx.enter_context(tc.tile_pool(name="psA", bufs=2, space="PSUM"))
        psZ = ctx.enter_context(tc.tile_pool(name="psZ", bufs=1, space="PSUM"))
        psU = ctx.enter_context(tc.tile_pool(name="psU", bufs=2, space="PSUM"))

        # ---- constants ----
        wqk_sb = [[], []]
        wv_sb = []
        for k, (k0, kw) in enumerate([(0, 128), (128, 128), (256, 2)]):
            for s in range(2):
                wt = const.tile([kw, 3 * 128], BF16, tag=f"wqk{s}_{k}")
                nc.sync.dma_start(out=wt, in_=wqkd[s, k0 : k0 + kw, :])
                wqk_sb[s].append(wt)
            vt = const.tile([kw, F], BF16, tag=f"wv{k}")
            nc.sync.dma_start(out=vt, in_=wvd[k0 : k0 + kw, :])
            wv_sb.append(vt)
        pos_sb = const.tile([128, 5, 6], F32, tag="pos")
        nc.sync.dma_start(out=pos_sb, in_=posd.rearrange("(t p) e -> p t e", p=128))
        ind8_sb = const.tile([128, BP, BP], BF16, tag="ind8")
        nc.vector.memset(ind8_sb, 0.0)
        for i in range(BP):
            nc.vector.memset(ind8_sb[:, i, i : i + 1], 1.0)
        onesb_sb = const.tile([128, 32], BF16, tag="onesb")
        nc.vector.memset(onesb_sb, 1.0)
        epssb = const.tile([32, 1], F32, tag="eps")
        nc.vector.memset(epssb, EPS)
        pb_sb = const.tile([B, OS], F32, tag="pb")
        nc.gpsimd.dma_start(out=pb_sb, in_=bcast_p(pbd[0, :], B))
        # HAM keep-alive source: dependency-free matmul fodder. The PE clock
        # gate only unthrottles after a fully-busy 4096-cycle window; these
        # extend each block's gapless stream past that and absorb the idle
        # that would otherwise re-throttle it.
        dsrc = const.tile([128, 512], BF16, tag="dsrc")
        nc.vector.memset(dsrc, 1.0)
        zpad = const.tile([1, 552], BF16, tag="zpad")
        nc.vector.memset(zpad, 0.0)
        for j in range(6):
            for ib in range(BP):
                nc.gpsimd.dma_start(
                    out=feat8d[j][ib, 4900:PADMH],
                    in_=bass.AP(tensor=zpad.tensor, offset=zpad.offset, ap=[[1, 1], [1, 92]]),
                )

        # ---- phase 1a: LN stats (x split across 4 DMA queues) ----
        qdma = [nc.sync, nc.gpsimd, nc.scalar]
        qrot = [0]

        def load_x(xt, s, i, k):
            # split each [128, N] tile into partition halves spread over the
            # DMA queues so startup isn't bound by one DMA engine's bandwidth
            for hh in range(2):
                q = qdma[qrot[0] % 3]
                qrot[0] += 1
                q.dma_start(
                    out=xt[hh * 64 : (hh + 1) * 64],
                    in_=xd[s][i, k * 128 + hh * 64 : k * 128 + (hh + 1) * 64, :],
                )

        isColT = []
        for s in range(2):
            psum_s = psA.tile([128, N], F32, tag="pA")
            psum_q = psA.tile([128, N], F32, tag="pA")
            for i in range(BP):
                for k in range(2):
                    xt = xres.tile([128, N], BF16, tag="x", bufs=6, name="xt")
                    load_x(xt, s, i, k)
                    xq = tmp.tile([128, N], BF16, tag="xsq")
                    nc.vector.tensor_mul(xq, xt, xt)
                    st = i == 0 and k == 0
                    for c0, cw in NCH:
                        nc.tensor.matmul(
                            psum_s[:BP, c0 : c0 + cw], ind8_sb[:, i, :], xt[:, c0 : c0 + cw],
                            start=st, stop=(i == BP - 1 and k == 1),
                        )
                        nc.tensor.matmul(
                            psum_q[:BP, c0 : c0 + cw], ind8_sb[:, i, :], xq[:, c0 : c0 + cw],
                            start=st, stop=(i == BP - 1 and k == 1),
                        )
            mean = stats.tile([32, N], F32, tag="mean")
            ex2 = stats.tile([32, N], F32, tag="ex2")
            nc.vector.tensor_scalar_mul(mean[:BP], psum_s[:BP], 1.0 / C)
            nc.vector.tensor_scalar_mul(ex2[:BP], psum_q[:BP], 1.0 / C)
            var = stats.tile([32, N], F32, tag="var")
            nc.vector.scalar_tensor_tensor(
                out=var[:BP], in0=mean[:BP], scalar=-1.0, in1=mean[:BP], op0=OP.mult, op1=OP.mult
            )
            nc.vector.tensor_add(var[:BP], var[:BP], ex2[:BP])
            sig = stats.tile([32, N], F32, tag="sig")
            nc.scalar.activation(out=sig[:BP], in_=var[:BP], func=AX.Sqrt, bias=epssb[:BP])
            isvf = stats.tile([32, N], F32, tag=f"isvf{s}")
            nc.vector.reciprocal(isvf[:BP], sig[:BP])
            negmu = stats.tile([32, N], BF16, tag="negmu")
            nc.vector.tensor_scalar_mul(negmu[:BP], mean[:BP], -1.0)
            sigb = stats.tile([32, N], BF16, tag="sigb")
            nc.vector.tensor_copy(sigb[:BP], sig[:BP])
            # stats round-trips go on the scalar queue (idle until attention)
            nc.scalar.dma_start(out=statsd[s, 0], in_=negmu[:BP])
            nc.scalar.dma_start(out=statsd[s, 1], in_=sigb[:BP])
            nc.scalar.dma_start(out=isvd[s], in_=isvf[:BP])
            zt_is = stats.tile([32, 18, 32], F32, tag="zt_is")
            nc.vector.transpose(out=zt_is, in_=isvf.rearrange("p (g q) -> p g q", q=32))
            ict = const.tile([128, 5, BP], F32, tag=f"iscol{s}")
            for a in range(4):
                ng = 5 if a < 2 else 4
                nc.vector.tensor_copy(
                    out=ict[32 * a : 32 * a + 32, 0:ng, :],
                    in_=zt_is[:, a : 18 : 4, 0:BP],
                )
            isColT.append(ict)

        # ---- phase 1b: QKV emitters (executed fused into the block loop) ----
        qs = {}
        ks = {}
        vp = {}

        def fetch_qkv(i, s):
            xe = stats.tile([2, N], BF16, tag="xe", bufs=6, name="xe")
            nc.scalar.dma_start(out=xe, in_=statsd[s, :, i, :])
            isb = tmp.tile([128, N], F32, tag="isb", bufs=6, name="isb")
            nc.scalar.dma_start(out=isb, in_=bcast_p(isvd[s, i, :], 128))
            xt0 = xres.tile([128, N], BF16, tag="x2", bufs=8, name="xt0")
            load_x(xt0, s, i, 0)
            xt1 = xres.tile([128, N], BF16, tag="x2", bufs=8, name="xt1")
            load_x(xt1, s, i, 1)
            return xe, isb, xt0, xt1

        def emit_qkv(i, s, fetched_in):
            xe, isb, xt0, xt1 = fetched_in
            rhs3 = [xt0, xt1, xe]
            pqs = []
            for h in range(H):
                pq = psA.tile([128, N], F32, tag="pA")
                for k in range(3):
                    for c0, cw in NCH:
                        nc.tensor.matmul(
                            pq[:, c0 : c0 + cw],
                            wqk_sb[s][k][:, h * 128 : (h + 1) * 128],
                            rhs3[k][:, c0 : c0 + cw],
                            start=(k == 0), stop=(k == 2),
                        )
                pqs.append(pq)
                if h >= 1:  # consume pq(h-1) so the pA pool never blocks
                    _qk_emit(nc, sb_qk, qs, ks, pqs[h - 1], isb, i, s, h - 1)
            pvs = []
            for nt in range(5):
                w = NT[nt]
                n0 = nt * 128
                pv = psA.tile([128, F], F32, tag="pA")
                for k in range(3):
                    nc.tensor.matmul(
                        pv[:w],
                        rhs3[k][:, n0 : n0 + w],
                        wv_sb[k],
                        start=(k == 0), stop=(k == 2),
                    )
                pvs.append(pv)
                if nt == 0:
                    _qk_emit(nc, sb_qk, qs, ks, pqs[2], isb, i, s, 2)
                if nt >= 2:
                    _vt_emit(nc, sb_vp, vp, pos_sb, isColT, pvs[nt - 2], i, s, nt - 2)
            _vt_emit(nc, sb_vp, vp, pos_sb, isColT, pvs[3], i, s, 3)
            _vt_emit(nc, sb_vp, vp, pos_sb, isColT, pvs[4], i, s, 4)

        # ---- phase 2: pipelined attention; half-gathers + proj overlap ----
        oacc = opool.tile([B, OS], F32, tag="oacc")
        nc.vector.memset(oacc, 0.0)

        GSZ = 13
        pw5 = []  # persistent proj weights for the split mh=5 projection

        def fetch_proj(mh, half=None):
            # issue the DMAs (feature transposes + weights) well before the
            # matmuls so the PE never head-blocks on them
            fts = []
            pws = []
            for gi, g0 in enumerate(range(0, 39, GSZ)):
                gsl = slice(g0 * 128, (g0 + GSZ) * 128)
                psl = slice(mh * PADMH + g0 * 128, mh * PADMH + (g0 + GSZ) * 128)
                if half is None:
                    ft = ftpool.tile([128, GSZ, B], BF16, tag="ft", bufs=6)
                    # two half-batch transposes into disjoint column halves;
                    # the column order is the permuted batch order BORDER
                    nc.sync.dma_start_transpose(
                        out=ft[:, :, 0 : B // 2], in_=featAG[mh][0][:, gsl]
                    )
                    nc.sync.dma_start_transpose(
                        out=ft[:, :, B // 2 : B], in_=featAG[mh][1][:, gsl]
                    )
                else:
                    ft = ftpool.tile([128, GSZ, B // 2], BF16, tag="fth")
                    nc.sync.dma_start_transpose(
                        out=ft, in_=featAG[mh][half][:, gsl]
                    )
                fts.append(ft)
                if half == 1:
                    pws.append(pw5[gi])
                else:
                    pw = ftpool.tile(
                        [128, GSZ, OS], BF16,
                        tag=("pw" if half is None else f"pw5_{gi}"),
                        bufs=(6 if half is None else 1),
                    )
                    nc.gpsimd.dma_start(
                        out=pw, in_=pwtd[psl].rearrange("(j p) o -> p j o", p=128)
                    )
                    if half == 0:
                        pw5.append(pw)
                    pws.append(pw)
            return fts, pws

        def emit_proj_mm(fetched, half=None):
            fts, pws = fetched
            for gi in range(3):
                opsum = psA.tile([64, OS], F32, tag="pA")
                for j in range(GSZ):
                    nc.tensor.matmul(
                        opsum[0 : 64 if half is None else 32],
                        fts[gi][:, j, :], pws[gi][:, j, :],
                        start=(j == 0), stop=(j == GSZ - 1),
                    )
                if half is None:
                    nc.vector.tensor_add(oacc, oacc, opsum)
                else:
                    hsl = slice(half * 32, half * 32 + 32)
                    nc.vector.tensor_add(oacc[hsl], oacc[hsl], opsum[0:32])

        def emit_gather(j, hf):
            nc.gpsimd.collective_compute(
                "AllGather",
                OP.bypass,
                replica_groups=[list(range(NCORES))],
                ins=[feat8d[j][hf * 4 : hf * 4 + 4, :]],
                outs=[featAG[j][hf][:]],
            )

        # fused block order: item-major for items 0-5 (QKV streams into the
        # pipeline), mh-major tail for items 6-7 (spreads the final gathers)
        blocks = [(m, h, i) for i in range(6) for m in range(2) for h in range(H)]
        blocks += [(m, h, i) for m in range(2) for h in range(H) for i in (6, 7)]
        fetched = {}

        class Blk:
            __slots__ = ("m", "h", "i", "et", "e2", "zr5", "rzr5", "zcp", "rzc",
                         "vpl", "us", "fps")

        def emit_pa_tile(b, nt):
            m, h, i = b.m, b.h, b.i
            qside = 1 - m
            w = NT[nt]
            n0 = nt * 128
            pa = psA.tile([128, N], F32, tag="pA")
            for c0, cw in NCH:
                nc.tensor.matmul(
                    pa[:w, c0 : c0 + cw],
                    qs[i, qside, h][:, n0 : n0 + w],
                    ks[i, m, h][:, c0 : c0 + cw],
                    start=True, stop=True,
                )
            et = epool.tile([128, N], BF16, tag="E")
            nc.scalar.activation(
                out=et[:w], in_=pa[:w], func=AX.Exp,
                accum_out=b.zr5[:w, nt : nt + 1],
            )
            b.et.append(et)
            # per-tile consumers: E^2, 1/Zr for this row-tile, and the
            # 1/Zr-scaled vp copy -- all ready well before up(b) next step
            _e2_emit(nc, e2pool, b, nt)
            nc.vector.reciprocal(b.rzr5[:w, nt : nt + 1], b.zr5[:w, nt : nt + 1])
            vpl = vlpool.tile([128, 72], BF16, tag="vpl")
            nc.vector.tensor_scalar_mul(
                vpl[:w, 0:70], vp[b.i, b.m, nt][:w, b.h, 0:70],
                b.rzr5[:w, nt : nt + 1],
            )
            b.vpl.append(vpl)

        def emit_zc(p):
            # streaming column sums: ones stationary, E moving (solid streams)
            p.zcp = psZ.tile([32, N], F32, tag="pZ")
            for nt in range(5):
                w = NT[nt]
                for c0, cw in NCH:
                    nc.tensor.matmul(
                        p.zcp[:, c0 : c0 + cw], onesb_sb[:w, :], p.et[nt][:w, c0 : c0 + cw],
                        start=(nt == 0), stop=(nt == 4),
                    )

        def emit_zcpost(p):
            # transpose/extract 1/Zc into per-partition layout [128, 5]
            zt = tmp.tile([32, 18, 32], F32, tag="zt")
            nc.vector.transpose(out=zt, in_=p.zcp.rearrange("p (g q) -> p g q", q=32))
            rz32 = zpool.tile([32, 18], F32, tag="rz32")
            nc.vector.reciprocal(rz32, zt[:, :, 0])
            p.rzc = zpool.tile([128, 8], F32, tag="rzc")
            for a in range(4):
                ng = 5 if a < 2 else 4
                nc.gpsimd.tensor_copy(
                    out=p.rzc[32 * a : 32 * a + 32, 0:ng],
                    in_=rz32[:, a : 18 : 4],
                )

        def make_up_us(p):
            ups = []

            def one_up(mc):
                w2 = NT[mc]
                up = psU.tile([128, 72], F32, tag="pU")
                for nt in range(5):
                    w = NT[nt]
                    nc.tensor.matmul(
                        up[:w2, 0:70],
                        p.e2[nt][:w, mc * 128 : mc * 128 + w2],
                        p.vpl[nt][:w, 0:70],
                        start=(nt == 0), stop=(nt == 4),
                    )
                ups.append(up)

            def one_us(mc):
                w2 = NT[mc]
                us = upool.tile([128, 72], BF16, tag="us", bufs=12)
                nc.vector.tensor_scalar_mul(
                    us[:w2, 0:70], ups[mc][:w2, 0:70], p.rzc[:w2, mc : mc + 1]
                )
                p.us.append(us)

            return one_up, one_us

        def emit_f(p):
            # f-chain runs one step after up/us: its deps are a step old, so
            # the PE never micro-waits on the us copies
            p.fps = psU.tile([128, 72], F32, tag="pU")
            for mc in range(5):
                w2 = NT[mc]
                nc.tensor.matmul(
                    p.fps[0:70, 0:70],
                    p.us[mc][:w2, 0:70],
                    vp[p.i, p.m, mc][:w2, p.h, 0:70],
                    start=(mc == 0), stop=(mc == 4),
                )

        def emit_fstore(p):
            mh = p.m * 3 + p.h
            fb = fpool.tile([70, 70], BF16, tag="fb")
            nc.vector.tensor_copy(out=fb, in_=p.fps[0:70, 0:70])
            nc.sync.dma_start(
                out=feat8d[mh][p.i, 0:4900].rearrange("(d e) -> d e", e=70),
                in_=fb,
            )
            if p.i == 3:
                emit_gather(mh, 0)
            elif p.i == BP - 1:
                emit_gather(mh, 1)

        # prologue QKV: items 0 and 1 fully before the first block
        qkv_pref = {}
        for ii in range(2):
            for s in range(2):
                qkv_pref[ii, s] = fetch_qkv(ii, s)
        for ii in range(2):
            for s in range(2):
                emit_qkv(ii, s, qkv_pref.pop((ii, s)))

        # per-step hooks in the mh-major tail: after tail-step t, proj work
        tail_fetch = {5: (5, 0), 6: (0, None), 8: (1, None), 10: (2, None)}
        tail_mm = {7: (5, 0), 8: (0, None), 10: (1, None)}

        p1 = None  # previous block (zc/up/us stage)
        p2 = None  # block before that (f stage)
        for bi, (m, h, i) in enumerate(blocks):
            tail_t = bi - 36  # >=0 inside the mh-major tail section
            b = Blk()
            b.m, b.h, b.i = m, h, i
            b.zr5 = zpool.tile([128, 8], F32, tag="zr5")
            b.rzr5 = zpool.tile([128, 8], F32, tag="rzr5")
            b.et = []
            b.e2 = []
            b.vpl = []
            b.us = []
            emit_pa_tile(b, 0)
            emit_pa_tile(b, 1)
            if p1 is not None:
                emit_zc(p1)  # PE: solid 576-col streams
            # dependency-free keep-alive matmuls: keep the gapless stream
            # long enough to cover a full HAM window and fill warm-state idle
            dmy = psU.tile([128, 512], F32, tag="pU", name="dmy")
            for _ in range(10):
                nc.tensor.matmul(dmy[0:32, :], onesb_sb, dsrc, start=True, stop=True)
            dscr = tmp.tile([1, 8], F32, tag="dscr", bufs=2, name="dscr")
            nc.vector.tensor_copy(dscr[0:1, 0:1], dmy[0:1, 0:1])
            emit_pa_tile(b, 2)
            if p2 is not None:
                emit_f(p2)  # PE shorts; all deps one step old
            if p1 is not None:
                one_up, one_us = make_up_us(p1)
                one_up(0)
                one_up(1)
            emit_pa_tile(b, 3)
            if p1 is not None:
                # zcpost late on DVE so it never head-blocks the e2/vpl
                # stream that next step's up matmuls depend on
                emit_zcpost(p1)
                one_us(0)
                one_us(1)
                one_up(2)
                one_us(2)
            emit_pa_tile(b, 4)
            if p1 is not None:
                one_up(3)
                one_us(3)
                one_up(4)
                one_us(4)
            if p2 is not None:
                emit_fstore(p2)  # DVE copy + sync DMA + gather hooks
            # streamed-in QKV for item i+2 (item-major section only)
            if i < 6 and bi < 36:
                step6 = bi % 6
                if i + 2 < 8:
                    if step6 == 0:
                        qkv_pref[i + 2, 0] = fetch_qkv(i + 2, 0)
                    elif step6 == 1:
                        emit_qkv(i + 2, 0, qkv_pref.pop((i + 2, 0)))
                    elif step6 == 2:
                        qkv_pref[i + 2, 1] = fetch_qkv(i + 2, 1)
                    elif step6 == 3:
                        emit_qkv(i + 2, 1, qkv_pref.pop((i + 2, 1)))
            if tail_t >= 0:
                if tail_t in tail_fetch:
                    j, hf = tail_fetch[tail_t]
                    fetched[j, hf] = fetch_proj(j, half=hf)
                if tail_t in tail_mm:
                    j, hf = tail_mm[tail_t]
                    emit_proj_mm(fetched.pop((j, hf)), half=hf)
            p2 = p1
            p1 = b

        # epilogue: drain the two pending blocks, then the remaining projs
        emit_f(p2)
        emit_zc(p1)
        emit_zcpost(p1)
        emit_fstore(p2)
        one_up, one_us = make_up_us(p1)
        for mc in range(5):
            one_up(mc)
            one_us(mc)
        emit_proj_mm(fetched.pop((2, None)))
        fetched[3] = fetch_proj(3)
        emit_f(p1)
        emit_fstore(p1)  # fires gather(5, 1)
        emit_proj_mm(fetched.pop(3))
        fetched[4] = fetch_proj(4)
        emit_proj_mm(fetched.pop(4))
        emit_proj_mm(fetch_proj(5, half=1), half=1)
        osb = opool.tile([B, OS], F32, tag="osb")
        nc.vector.tensor_add(osb, oacc, pb_sb)
        nc.vector.tensor_scalar_max(osb, osb, 0.0)
        nc.sync.dma_start(out=outd[:], in_=osb)

    nc.compile()
    return nc


def _e2_emit(nc, e2pool, b, nt):
    w = NT[nt]
    e2 = e2pool.tile([128, N], BF16, tag="E2", name="e2")
    nc.vector.tensor_mul(e2[:w], b.et[nt][:w], b.et[nt][:w])
    b.e2.append(e2)


def _qk_emit(nc, sb_qk, qs, ks, pq, isb, i, s, h):
    # rotating ring: item i's tiles die once its 6 blocks are done
    qk = sb_qk.tile([128, N], BF16, tag=f"qk{s}_{h}", bufs=4, name="qk")
    nc.vector.tensor_mul(qk, pq, isb)
    if s == 0:
        ks[i, s, h] = qk[0:64, :]
        qs[i, s, h] = qk[64:128, :]
    else:
        qs[i, s, h] = qk[0:64, :]
        ks[i, s, h] = qk[64:128, :]


def _vt_emit(nc, sb_vp, vp, pos_sb, isColT, pv, i, s, nt):
    w = NT[nt]
    vt = sb_vp.tile([128, 3, 72], mybir.dt.bfloat16, tag=f"vp{s}_{nt}", bufs=4, name="vt")
    nc.vector.tensor_scalar_mul(
        vt[:w, :, 0:64],
        pv[:w, 0:F].rearrange("p (a b) -> p a b", b=64),
        isColT[s][:w, nt, i : i + 1],
    )
    ps = pos_sb[:w, nt, :]
    nc.gpsimd.tensor_copy(
        out=vt[:w, :, 64:70],
        in_=bass.AP(tensor=ps.tensor, offset=ps.offset,
                    ap=[ps.ap[0], [0, 3], ps.ap[-1]]),
    )
    vp[i, s, nt] = vt


def kernel(x1, x2, ln_w, ln_b, qkv_w, proj_w, proj_b):
    wqk, wv, pos_pad, pwt = _host_prep(ln_w, ln_b, qkv_w, proj_w, proj_b)
    xs1 = np.ascontiguousarray(x1.reshape(B, C, N)).astype(ml_dtypes.bfloat16)
    xs2 = np.ascontiguousarray(x2.reshape(B, C, N)).astype(ml_dtypes.bfloat16)
    nc = _build()
    in_maps = []
    for r in range(NCORES):
        in_maps.append(
            {
                "x1s": xs1[r * BP : (r + 1) * BP],
                "x2s": xs2[r * BP : (r + 1) * BP],
                "wqk": wqk,
                "wv": wv,
                "pos": pos_pad,
                "pwt": np.ascontiguousarray(pwt[:, r * OS : (r + 1) * OS]),
                "pb": np.ascontiguousarray(proj_b[None, r * OS : (r + 1) * OS]).astype(np.float32),
            }
        )
    import os

    trace = bool(os.environ.get("BASS_TRACE"))
    res = run_bass_kernel_spmd(nc, in_maps, core_ids=list(range(NCORES)), trace=trace)
    if res.exec_time_ns is not None:
        print(f"HW exec time: {res.exec_time_ns} ns")
    if res.instructions_and_trace:
        print("trace path:", res.instructions_and_trace[1])
    # per-core outputs are in the permuted (half-gather) batch order
    out = np.empty((B, 512), np.float32)
    for r in range(NCORES):
        out[BORDER, r * OS : (r + 1) * OS] = res.results[r]["out"]
    return out


if __name__ == "__main__":
    rng = np.random.default_rng(0)
    ins = {
        "x1": rng.standard_normal((B, C, HG, WG), dtype=np.float32),
        "x2": rng.standard_normal((B, C, HG, WG), dtype=np.float32),
        "ln_w": np.ones(C, np.float32),
        "ln_b": np.zeros(C, np.float32),
        "qkv_w": (rng.standard_normal((3 * F, C)) * C**-0.5).astype(np.float32),
        "proj_w": (rng.standard_normal((512, 6 * 4900)) * (6 * 4900) ** -0.5).astype(np.float32),
        "proj_b": np.zeros(512, np.float32),
    }
    print(kernel(**ins).shape)


# revision 56
# speedup vs baseline: 1.0659x; 1.0659x over previous
"""Trainium2 Bass kernel for nn_EssentialMatixModule.

Dual-softmax cross-attention (LoFTR-style) + bilinear feature + projection.
Data-parallel over batch across 8 cores; proj output-sharded with chunked
AllGathers of the (bf16) feature matrix overlapping the attention phase.

Structure: LN stats (x split over 3 DMA queues), then one fused
software-pipelined loop: per-item QKV streams into the attention blocks
(item-major order, mh-major tail), each block is a 3-stage pipeline
(scores+exp -> colsums+up/us -> f), with E^2 / 1/Zr / vpl emitted
per-row-tile right after each exp so every PE matmul's dependencies are
at least one block old.  Column sums stream through the PE with a
stationary ones vector; 1/sigma is broadcast by DMA (f32 exact).
Feature chunks AllGather in half-batches (permuted batch order, undone
on the host) and the output projection is prefetched and interleaved so
the serial tail is short.  The scalar queue carries only activations
during attention.
"""

import sys

sys.path.insert(0, "/opt/trn_rl_repo")

from contextlib import ExitStack

import ml_dtypes
import numpy as np

import concourse.bass as bass
import concourse.tile as tile
from concourse import bacc, mybir
from concourse.bass_utils import run_bass_kernel_spmd

B, C, HG, WG = 64, 256, 24, 24
N = HG * WG  # 576
H, HD = 3, 64
F = H * HD  # 192
SCALE = HD**-0.5
EPS = 1e-5
NCORES = 8
BP = B // NCORES  # 8 items per core
NT = [128, 128, 128, 128, 64]  # token tiles (sum=576)
# free-dim chunks for N=576 psum; 64-chunk first so each matmul pair ends
# with a 512-col stream that hides the next LDWEIGHTS
NCH = [(512, 64), (0, 512)]
DE = 70  # hd + 6 pos dims
PADMH = 4992  # 39*128, per-(map,head) padded feat block
DIMS = 6 * PADMH  # 29952
OS = 512 // NCORES  # 64 output cols per core
F32 = mybir.dt.float32
BF16 = mybir.dt.bfloat16
AX = mybir.ActivationFunctionType
OP = mybir.AluOpType

# half-batch gather row order: B' = [items 0-3 of each core, items 4-7 of each core]
BORDER = np.array(
    [8 * c + i for i in (0,) for c in range(0)]  # placeholder, built below
)
_rows = []
for c in range(NCORES):
    for i in range(4):
        _rows.append(8 * c + i)
for c in range(NCORES):
    for i in range(4, 8):
        _rows.append(8 * c + i)
BORDER = np.array(_rows)  # BORDER[r] = original batch index of permuted row r


def _host_prep(ln_w, ln_b, qkv_w, proj_w, proj_b):
    ln_w = ln_w.astype(np.float64)
    ln_b = ln_b.astype(np.float64)
    qw = qkv_w.astype(np.float64)
    Wp = qw * ln_w[None, :]  # [576, C]
    r = Wp.sum(axis=1)  # [576]
    t = qw @ ln_b  # [576]

    # per-side packing: side0 tiles hold [k_h; q_h], side1 [q_h; k_h] so the
    # attention matmul operands always share a partition base
    def col(fsl, scale):
        return np.concatenate([Wp[fsl] * scale, (r[fsl] * scale)[:, None],
                               (t[fsl] * scale)[:, None]], axis=1).T

    wqk = np.zeros((2, C + 2, 3 * 128), np.float32)
    for h in range(H):
        qr = slice(h * HD, (h + 1) * HD)
        kr = slice(F + h * HD, F + (h + 1) * HD)
        qcols = col(qr, SCALE)  # [C+2, 64]
        kcols = col(kr, 1.0)
        wqk[0, :, h * 128 : h * 128 + 64] = kcols
        wqk[0, :, h * 128 + 64 : h * 128 + 128] = qcols
        wqk[1, :, h * 128 : h * 128 + 64] = qcols
        wqk[1, :, h * 128 + 64 : h * 128 + 128] = kcols
    wqk = wqk.astype(ml_dtypes.bfloat16)

    wv = np.zeros((C + 2, F), np.float32)
    wv[:C] = Wp[2 * F :].T
    wv[C] = r[2 * F :]
    wv[C + 1] = t[2 * F :]
    wv = wv.astype(ml_dtypes.bfloat16)

    ys = np.linspace(-1.0, 1.0, HG)
    xs = np.linspace(-1.0, 1.0, WG)
    p3 = np.tile(ys, WG)
    p4 = np.repeat(xs, HG)
    pos = np.stack([p3 * p3, p4 * p4, p3 * p4, p3, p4, np.ones_like(p3)], axis=1)
    pos_pad = np.zeros((640, 6), np.float32)
    pos_pad[:N] = pos

    pwt = np.zeros((DIMS, 512), np.float32)
    for mh in range(6):
        blk = proj_w[:, mh * 4900 : (mh + 1) * 4900]  # [512, 4900]
        pwt[mh * PADMH : mh * PADMH + 4900] = blk.T
    pwt = pwt.astype(ml_dtypes.bfloat16)
    return wqk, wv, pos_pad, pwt


def _build():
    nc = bacc.Bacc()
    x1d = nc.declare_dram_parameter("x1s", [BP, C, N], BF16, isOutput=False)
    x2d = nc.declare_dram_parameter("x2s", [BP, C, N], BF16, isOutput=False)
    wqkd = nc.declare_dram_parameter("wqk", [2, C + 2, 3 * 128], BF16, isOutput=False)
    wvd = nc.declare_dram_parameter("wv", [C + 2, F], BF16, isOutput=False)
    posd = nc.declare_dram_parameter("pos", [640, 6], F32, isOutput=False)
    pwtd = nc.declare_dram_parameter("pwt", [DIMS, OS], BF16, isOutput=False)
    pbd = nc.declare_dram_parameter("pb", [1, OS], F32, isOutput=False)
    outd = nc.declare_dram_parameter("out", [B, OS], F32, isOutput=True)
    statsd = nc.dram_tensor("statsd", [2, 2, BP, N], BF16)  # (negmu, sigma)
    isvd = nc.dram_tensor("isvd", [2, BP, N], F32)  # 1/sigma rows, f32
    feat8d = [nc.dram_tensor(f"feat8_{j}", [BP, PADMH], BF16) for j in range(6)]
    # two half-batch gather outputs per mh chunk (items 0-3, items 4-7)
    featAG = [
        [
            nc.dram_tensor(f"featAG_{j}_{hf}", [B // 2, PADMH], BF16, addr_space="Shared")
            for hf in range(2)
        ]
        for j in range(6)
    ]
    xd = [x1d, x2d]

    def bcast_p(sl, p):
        return bass.AP(tensor=sl.tensor, offset=sl.offset, ap=[[0, p]] + list(sl.ap))

    with ExitStack() as ctx:
        tc = ctx.enter_context(tile.TileContext(nc))
        const = ctx.enter_context(tc.tile_pool(name="const", bufs=1))
        xres = ctx.enter_context(tc.tile_pool(name="xres", bufs=1))
        stats = ctx.enter_context(tc.tile_pool(name="stats", bufs=1))
        tmp = ctx.enter_context(tc.tile_pool(name="tmp", bufs=2))
        sb_qk = ctx.enter_context(tc.tile_pool(name="sbqk", bufs=1))
        sb_vp = ctx.enter_context(tc.tile_pool(name="sbvp", bufs=1))
        epool = ctx.enter_context(tc.tile_pool(name="epool", bufs=11))
        e2pool = ctx.enter_context(tc.tile_pool(name="e2pool", bufs=11))
        zpool = ctx.enter_context(tc.tile_pool(name="zpool", bufs=3))
        upool = ctx.enter_context(tc.tile_pool(name="upool", bufs=8))
        vlpool = ctx.enter_context(tc.tile_pool(name="vlpool", bufs=12))
        fpool = ctx.enter_context(tc.tile_pool(name="fpool", bufs=3))
        ftpool = ctx.enter_context(tc.tile_pool(name="ftpool", bufs=3))
        opool = ctx.enter_context(tc.tile_pool(name="opool", bufs=2))
        # PSUM: tag pA [128,576]x2 = 4 banks; pZ [128,576]x1 = 2; pU [128,192]x2 = 2
        psA = ctx.enter_context(tc.tile_pool(name="psA", bufs=2, space="PSUM"))
        psZ = ctx.enter_context(tc.tile_pool(name="psZ", bufs=1, space="PSUM"))
        psU = ctx.enter_context(tc.tile_pool(name="psU", bufs=2, space="PSUM"))

        # ---- constants ----
        wqk_sb = [[], []]
        wv_sb = []
        for k, (k0, kw) in enumerate([(0, 128), (128, 128), (256, 2)]):
            for s in range(2):
                wt = const.tile([kw, 3 * 128], BF16, tag=f"wqk{s}_{k}")
                nc.sync.dma_start(out=wt, in_=wqkd[s, k0 : k0 + kw, :])
                wqk_sb[s].append(wt)
            vt = const.tile([kw, F], BF16, tag=f"wv{k}")
            nc.sync.dma_start(out=vt, in_=wvd[k0 : k0 + kw, :])
            wv_sb.append(vt)
        pos_sb = const.tile([128, 5, 6], F32, tag="pos")
        nc.sync.dma_start(out=pos_sb, in_=posd.rearrange("(t p) e -> p t e", p=128))
        ind8_sb = const.tile([128, BP, BP], BF16, tag="ind8")
        nc.vector.memset(ind8_sb, 0.0)
        for i in range(BP):
            nc.vector.memset(ind8_sb[:, i, i : i + 1], 1.0)
        onesb_sb = const.tile([128, 32], BF16, tag="onesb")
        nc.vector.memset(onesb_sb, 1.0)
        epssb = const.tile([32, 1], F32, tag="eps")
        nc.vector.memset(epssb, EPS)
        pb_sb = const.tile([B, OS], F32, tag="pb")
        nc.gpsimd.dma_start(out=pb_sb, in_=bcast_p(pbd[0, :], B))
        # HAM keep-alive source: dependency-free matmul fodder. The PE clock
        # gate only unthrottles after a fully-busy 4096-cycle window; these
        # extend each block's gapless stream past that and absorb the idle
        # that would otherwise re-throttle it.
        dsrc = const.tile([128, 512], BF16, tag="dsrc")
        nc.vector.memset(dsrc, 1.0)
        zpad = const.tile([1, 552], BF16, tag="zpad")
        nc.vector.memset(zpad, 0.0)
        for j in range(6):
            for ib in range(BP):
                nc.gpsimd.dma_start(
                    out=feat8d[j][ib, 4900:PADMH],
                    in_=bass.AP(tensor=zpad.tensor, offset=zpad.offset, ap=[[1, 1], [1, 92]]),
                )

        # ---- phase 1a: LN stats (x split across 4 DMA queues) ----
        qdma = [nc.sync, nc.gpsimd, nc.scalar]
        qrot = [0]

        def load_x(xt, s, i, k):
            # split each [128, N] tile into partition halves spread over the
            # DMA queues so startup isn't bound by one DMA engine's bandwidth
            for hh in range(2):
                q = qdma[qrot[0] % 3]
                qrot[0] += 1
                q.dma_start(
                    out=xt[hh * 64 : (hh + 1) * 64],
                    in_=xd[s][i, k * 128 + hh * 64 : k * 128 + (hh + 1) * 64, :],
                )

        isColT = []
        for s in range(2):
            psum_s = psA.tile([128, N], F32, tag="pA")
            psum_q = psA.tile([128, N], F32, tag="pA")
            for i in range(BP):
                for k in range(2):
                    xt = xres.tile([128, N], BF16, tag="x", bufs=6, name="xt")
                    load_x(xt, s, i, k)
                    xq = tmp.tile([128, N], BF16, tag="xsq")
                    nc.vector.tensor_mul(xq, xt, xt)
                    st = i == 0 and k == 0
                    for c0, cw in NCH:
                        nc.tensor.matmul(
                            psum_s[:BP, c0 : c0 + cw], ind8_sb[:, i, :], xt[:, c0 : c0 + cw],
                            start=st, stop=(i == BP - 1 and k == 1),
                        )
                        nc.tensor.matmul(
                            psum_q[:BP, c0 : c0 + cw], ind8_sb[:, i, :], xq[:, c0 : c0 + cw],
                            start=st, stop=(i == BP - 1 and k == 1),
                        )
            mean = stats.tile([32, N], F32, tag="mean")
            ex2 = stats.tile([32, N], F32, tag="ex2")
            nc.vector.tensor_scalar_mul(mean[:BP], psum_s[:BP], 1.0 / C)
            nc.vector.tensor_scalar_mul(ex2[:BP], psum_q[:BP], 1.0 / C)
            var = stats.tile([32, N], F32, tag="var")
            nc.vector.scalar_tensor_tensor(
                out=var[:BP], in0=mean[:BP], scalar=-1.0, in1=mean[:BP], op0=OP.mult, op1=OP.mult
            )
            nc.vector.tensor_add(var[:BP], var[:BP], ex2[:BP])
            sig = stats.tile([32, N], F32, tag="sig")
            nc.scalar.activation(out=sig[:BP], in_=var[:BP], func=AX.Sqrt, bias=epssb[:BP])
            isvf = stats.tile([32, N], F32, tag=f"isvf{s}")
            nc.vector.reciprocal(isvf[:BP], sig[:BP])
            negmu = stats.tile([32, N], BF16, tag="negmu")
            nc.vector.tensor_scalar_mul(negmu[:BP], mean[:BP], -1.0)
            sigb = stats.tile([32, N], BF16, tag="sigb")
            nc.vector.tensor_copy(sigb[:BP], sig[:BP])
            # stats round-trips go on the scalar queue (idle until attention)
            nc.scalar.dma_start(out=statsd[s, 0], in_=negmu[:BP])
            nc.scalar.dma_start(out=statsd[s, 1], in_=sigb[:BP])
            nc.scalar.dma_start(out=isvd[s], in_=isvf[:BP])
            zt_is = stats.tile([32, 18, 32], F32, tag="zt_is")
            nc.vector.transpose(out=zt_is, in_=isvf.rearrange("p (g q) -> p g q", q=32))
            ict = const.tile([128, 5, BP], F32, tag=f"iscol{s}")
            for a in range(4):
                ng = 5 if a < 2 else 4
                nc.vector.tensor_copy(
                    out=ict[32 * a : 32 * a + 32, 0:ng, :],
                    in_=zt_is[:, a : 18 : 4, 0:BP],
                )
            isColT.append(ict)

        # ---- phase 1b: QKV emitters (executed fused into the block loop) ----
        qs = {}
        ks = {}
        vp = {}

        def fetch_qkv(i, s):
            xe = stats.tile([2, N], BF16, tag="xe", bufs=6, name="xe")
            nc.scalar.dma_start(out=xe, in_=statsd[s, :, i, :])
            isb = tmp.tile([128, N], F32, tag="isb", bufs=6, name="isb")
            nc.scalar.dma_start(out=isb, in_=bcast_p(isvd[s, i, :], 128))
            xt0 = xres.tile([128, N], BF16, tag="x2", bufs=8, name="xt0")
            load_x(xt0, s, i, 0)
            xt1 = xres.tile([128, N], BF16, tag="x2", bufs=8, name="xt1")
            load_x(xt1, s, i, 1)
            return xe, isb, xt0, xt1

        def emit_qkv(i, s, fetched_in):
            xe, isb, xt0, xt1 = fetched_in
            rhs3 = [xt0, xt1, xe]
            pqs = []
            for h in range(H):
                pq = psA.tile([128, N], F32, tag="pA")
                for k in range(3):
                    for c0, cw in NCH:
                        nc.tensor.matmul(
                            pq[:, c0 : c0 + cw],
                            wqk_sb[s][k][:, h * 128 : (h + 1) * 128],
                            rhs3[k][:, c0 : c0 + cw],
                            start=(k == 0), stop=(k == 2),
                        )
                pqs.append(pq)
                if h >= 1:  # consume pq(h-1) so the pA pool never blocks
                    _qk_emit(nc, sb_qk, qs, ks, pqs[h - 1], isb, i, s, h - 1)
            pvs = []
            for nt in range(5):
                w = NT[nt]
                n0 = nt * 128
                pv = psA.tile([128, F], F32, tag="pA")
                for k in range(3):
                    nc.tensor.matmul(
                        pv[:w],
                        rhs3[k][:, n0 : n0 + w],
                        wv_sb[k],
                        start=(k == 0), stop=(k == 2),
                    )
                pvs.append(pv)
                if nt == 0:
                    _qk_emit(nc, sb_qk, qs, ks, pqs[2], isb, i, s, 2)
                if nt >= 2:
                    _vt_emit(nc, sb_vp, vp, pos_sb, isColT, pvs[nt - 2], i, s, nt - 2)
            _vt_emit(nc, sb_vp, vp, pos_sb, isColT, pvs[3], i, s, 3)
            _vt_emit(nc, sb_vp, vp, pos_sb, isColT, pvs[4], i, s, 4)

        # ---- phase 2: pipelined attention; half-gathers + proj overlap ----
        oacc = opool.tile([B, OS], F32, tag="oacc")
        nc.vector.memset(oacc, 0.0)

        GSZ = 13
        pw5 = []  # persistent proj weights for the split mh=5 projection

        def fetch_proj(mh, half=None):
            # issue the DMAs (feature transposes + weights) well before the
            # matmuls so the PE never head-blocks on them
            fts = []
            pws = []
            for gi, g0 in enumerate(range(0, 39, GSZ)):
                gsl = slice(g0 * 128, (g0 + GSZ) * 128)
                psl = slice(mh * PADMH + g0 * 128, mh * PADMH + (g0 + GSZ) * 128)
                if half is None:
                    ft = ftpool.tile([128, GSZ, B], BF16, tag="ft", bufs=6)
                    # two half-batch transposes into disjoint column halves;
                    # the column order is the permuted batch order BORDER
                    nc.sync.dma_start_transpose(
                        out=ft[:, :, 0 : B // 2], in_=featAG[mh][0][:, gsl]
                    )
                    nc.sync.dma_start_transpose(
                        out=ft[:, :, B // 2 : B], in_=featAG[mh][1][:, gsl]
                    )
                else:
                    ft = ftpool.tile([128, GSZ, B // 2], BF16, tag="fth")
                    nc.sync.dma_start_transpose(
                        out=ft, in_=featAG[mh][half][:, gsl]
                    )
                fts.append(ft)
                if half == 1:
                    pws.append(pw5[gi])
                else:
                    pw = ftpool.tile(
                        [128, GSZ, OS], BF16,
                        tag=("pw" if half is None else f"pw5_{gi}"),
                        bufs=(6 if half is None else 1),
                    )
                    nc.gpsimd.dma_start(
                        out=pw, in_=pwtd[psl].rearrange("(j p) o -> p j o", p=128)
                    )
                    if half == 0:
                        pw5.append(pw)
                    pws.append(pw)
            return fts, pws

        def emit_proj_mm(fetched, half=None):
            fts, pws = fetched
            for gi in range(3):
                opsum = psA.tile([64, OS], F32, tag="pA")
                for j in range(GSZ):
                    nc.tensor.matmul(
                        opsum[0 : 64 if half is None else 32],
                        fts[gi][:, j, :], pws[gi][:, j, :],
                        start=(j == 0), stop=(j == GSZ - 1),
                    )
                if half is None:
                    nc.vector.tensor_add(oacc, oacc, opsum)
                else:
                    hsl = slice(half * 32, half * 32 + 32)
                    nc.vector.tensor_add(oacc[hsl], oacc[hsl], opsum[0:32])

        def emit_gather(j, hf):
            nc.gpsimd.collective_compute(
                "AllGather",
                OP.bypass,
                replica_groups=[list(range(NCORES))],
                ins=[feat8d[j][hf * 4 : hf * 4 + 4, :]],
                outs=[featAG[j][hf][:]],
            )

        # fused block order: item-major for items 0-5 (QKV streams into the
        # pipeline), mh-major tail for items 6-7 (spreads the final gathers)
        blocks = [(m, h, i) for i in range(6) for m in range(2) for h in range(H)]
        blocks += [(m, h, i) for m in range(2) for h in range(H) for i in (6, 7)]
        fetched = {}

        class Blk:
            __slots__ = ("m", "h", "i", "et", "e2", "zr5", "rzr5", "zcp", "rzc",
                         "vpl", "us", "fps")

        def emit_pa_tile(b, nt):
            m, h, i = b.m, b.h, b.i
            qside = 1 - m
            w = NT[nt]
            n0 = nt * 128
            pa = psA.tile([128, N], F32, tag="pA")
            for c0, cw in NCH:
                nc.tensor.matmul(
                    pa[:w, c0 : c0 + cw],
                    qs[i, qside, h][:, n0 : n0 + w],
                    ks[i, m, h][:, c0 : c0 + cw],
                    start=True, stop=True,
                )
            et = epool.tile([128, N], BF16, tag="E")
            nc.scalar.activation(
                out=et[:w], in_=pa[:w], func=AX.Exp,
                accum_out=b.zr5[:w, nt : nt + 1],
            )
            b.et.append(et)
            # per-tile consumers: E^2, 1/Zr for this row-tile, and the
            # 1/Zr-scaled vp copy -- all ready well before up(b) next step
            _e2_emit(nc, e2pool, b, nt)
            nc.vector.reciprocal(b.rzr5[:w, nt : nt + 1], b.zr5[:w, nt : nt + 1])
            vpl = vlpool.tile([128, 72], BF16, tag="vpl")
            nc.vector.tensor_scalar_mul(
                vpl[:w, 0:70], vp[b.i, b.m, nt][:w, b.h, 0:70],
                b.rzr5[:w, nt : nt + 1],
            )
            b.vpl.append(vpl)

        def emit_zc(p):
            # streaming column sums: ones stationary, E moving (solid streams)
            p.zcp = psZ.tile([32, N], F32, tag="pZ")
            for nt in range(5):
                w = NT[nt]
                for c0, cw in NCH:
                    nc.tensor.matmul(
                        p.zcp[:, c0 : c0 + cw], onesb_sb[:w, :], p.et[nt][:w, c0 : c0 + cw],
                        start=(nt == 0), stop=(nt == 4),
                    )

        def emit_zcpost(p):
            # transpose/extract 1/Zc into per-partition layout [128, 5]
            zt = tmp.tile([32, 18, 32], F32, tag="zt")
            nc.vector.transpose(out=zt, in_=p.zcp.rearrange("p (g q) -> p g q", q=32))
            rz32 = zpool.tile([32, 18], F32, tag="rz32")
            nc.vector.reciprocal(rz32, zt[:, :, 0])
            p.rzc = zpool.tile([128, 8], F32, tag="rzc")
            for a in range(4):
                ng = 5 if a < 2 else 4
                nc.gpsimd.tensor_copy(
                    out=p.rzc[32 * a : 32 * a + 32, 0:ng],
                    in_=rz32[:, a : 18 : 4],
                )

        def make_up_us(p):
            ups = []

            def one_up(mc):
                w2 = NT[mc]
                up = psU.tile([128, 72], F32, tag="pU")
                for nt in range(5):
                    w = NT[nt]
                    nc.tensor.matmul(
                        up[:w2, 0:70],
                        p.e2[nt][:w, mc * 128 : mc * 128 + w2],
                        p.vpl[nt][:w, 0:70],
                        start=(nt == 0), stop=(nt == 4),
                    )
                ups.append(up)

            def one_us(mc):
                w2 = NT[mc]
                us = upool.tile([128, 72], BF16, tag="us", bufs=12)
                nc.vector.tensor_scalar_mul(
                    us[:w2, 0:70], ups[mc][:w2, 0:70], p.rzc[:w2, mc : mc + 1]
                )
                p.us.append(us)

            return one_up, one_us

        def emit_f(p):
            # f-chain runs one step after up/us: its deps are a step old, so
            # the PE never micro-waits on the us copies
            p.fps = psU.tile([128, 72], F32, tag="pU")
            for mc in range(5):
                w2 = NT[mc]
                nc.tensor.matmul(
                    p.fps[0:70, 0:70],
                    p.us[mc][:w2, 0:70],
                    vp[p.i, p.m, mc][:w2, p.h, 0:70],
                    start=(mc == 0), stop=(mc == 4),
                )

        def emit_fstore(p):
            mh = p.m * 3 + p.h
            fb = fpool.tile([70, 70], BF16, tag="fb")
            nc.vector.tensor_copy(out=fb, in_=p.fps[0:70, 0:70])
            nc.sync.dma_start(
                out=feat8d[mh][p.i, 0:4900].rearrange("(d e) -> d e", e=70),
                in_=fb,
            )
            if p.i == 3:
                emit_gather(mh, 0)
            elif p.i == BP - 1:
                emit_gather(mh, 1)

        # prologue QKV: items 0 and 1 fully before the first block
        qkv_pref = {}
        for ii in range(2):
            for s in range(2):
                qkv_pref[ii, s] = fetch_qkv(ii, s)
        for ii in range(2):
            for s in range(2):
                emit_qkv(ii, s, qkv_pref.pop((ii, s)))

        # per-step hooks in the mh-major tail: after tail-step t, proj work
        tail_fetch = {5: (5, 0), 6: (0, None), 8: (1, None), 10: (2, None)}
        tail_mm = {7: (5, 0), 8: (0, None), 10: (1, None)}

        p1 = None  # previous block (zc/up/us stage)
        p2 = None  # block before that (f stage)
        for bi, (m, h, i) in enumerate(blocks):
            tail_t = bi - 36  # >=0 inside the mh-major tail section
            b = Blk()
            b.m, b.h, b.i = m, h, i
            b.zr5 = zpool.tile([128, 8], F32, tag="zr5")
            b.rzr5 = zpool.tile([128, 8], F32, tag="rzr5")
            b.et = []
            b.e2 = []
            b.vpl = []
            b.us = []
            emit_pa_tile(b, 0)
            emit_pa_tile(b, 1)
            if p1 is not None:
                emit_zc(p1)  # PE: solid 576-col streams
            # dependency-free keep-alive matmuls: keep the gapless stream
            # long enough to cover a full HAM window and fill warm-state idle
            dmy = psU.tile([128, 512], F32, tag="pU", name="dmy")
            for _ in range(5):
                nc.tensor.matmul(dmy[0:32, :], onesb_sb, dsrc, start=True, stop=True)
            dscr = tmp.tile([1, 8], F32, tag="dscr", bufs=2, name="dscr")
            nc.vector.tensor_copy(dscr[0:1, 0:1], dmy[0:1, 0:1])
            emit_pa_tile(b, 2)
            if p2 is not None:
                emit_f(p2)  # PE shorts; all deps one step old
            if p1 is not None:
                one_up, one_us = make_up_us(p1)
                one_up(0)
                one_up(1)
            emit_pa_tile(b, 3)
            if p1 is not None:
                # zcpost late on DVE so it never head-blocks the e2/vpl
                # stream that next step's up matmuls depend on
                emit_zcpost(p1)
                one_us(0)
                one_us(1)
                one_up(2)
                one_us(2)
            emit_pa_tile(b, 4)
            if p1 is not None:
                one_up(3)
                one_us(3)
                one_up(4)
                one_us(4)
            if p2 is not None:
                emit_fstore(p2)  # DVE copy + sync DMA + gather hooks
            # streamed-in QKV for item i+2 (item-major section only)
            if i < 6 and bi < 36:
                step6 = bi % 6
                if i + 2 < 8:
                    if step6 == 0:
                        qkv_pref[i + 2, 0] = fetch_qkv(i + 2, 0)
                    elif step6 == 1:
                        emit_qkv(i + 2, 0, qkv_pref.pop((i + 2, 0)))
                    elif step6 == 2:
                        qkv_pref[i + 2, 1] = fetch_qkv(i + 2, 1)
                    elif step6 == 3:
                        emit_qkv(i + 2, 1, qkv_pref.pop((i + 2, 1)))
            if tail_t >= 0:
                if tail_t in tail_fetch:
                    j, hf = tail_fetch[tail_t]
                    fetched[j, hf] = fetch_proj(j, half=hf)
                if tail_t in tail_mm:
                    j, hf = tail_mm[tail_t]
                    emit_proj_mm(fetched.pop((j, hf)), half=hf)
            p2 = p1
            p1 = b

        # epilogue: drain the two pending blocks, then the remaining projs
        emit_f(p2)
        emit_zc(p1)
        emit_zcpost(p1)
        emit_fstore(p2)
        one_up, one_us = make_up_us(p1)
        for mc in range(5):
            one_up(mc)
            one_us(mc)
        emit_proj_mm(fetched.pop((2, None)))
        fetched[3] = fetch_proj(3)
        emit_f(p1)
        emit_fstore(p1)  # fires gather(5, 1)
        emit_proj_mm(fetched.pop(3))
        fetched[4] = fetch_proj(4)
        emit_proj_mm(fetched.pop(4))
        emit_proj_mm(fetch_proj(5, half=1), half=1)
        osb = opool.tile([B, OS], F32, tag="osb")
        nc.vector.tensor_add(osb, oacc, pb_sb)
        nc.vector.tensor_scalar_max(osb, osb, 0.0)
        nc.sync.dma_start(out=outd[:], in_=osb)

    nc.compile()
    return nc


def _e2_emit(nc, e2pool, b, nt):
    w = NT[nt]
    e2 = e2pool.tile([128, N], BF16, tag="E2", name="e2")
    nc.vector.tensor_mul(e2[:w], b.et[nt][:w], b.et[nt][:w])
    b.e2.append(e2)


def _qk_emit(nc, sb_qk, qs, ks, pq, isb, i, s, h):
    # rotating ring: item i's tiles die once its 6 blocks are done
    qk = sb_qk.tile([128, N], BF16, tag=f"qk{s}_{h}", bufs=4, name="qk")
    nc.vector.tensor_mul(qk, pq, isb)
    if s == 0:
        ks[i, s, h] = qk[0:64, :]
        qs[i, s, h] = qk[64:128, :]
    else:
        qs[i, s, h] = qk[0:64, :]
        ks[i, s, h] = qk[64:128, :]


def _vt_emit(nc, sb_vp, vp, pos_sb, isColT, pv, i, s, nt):
    w = NT[nt]
    vt = sb_vp.tile([128, 3, 72], mybir.dt.bfloat16, tag=f"vp{s}_{nt}", bufs=4, name="vt")
    nc.vector.tensor_scalar_mul(
        vt[:w, :, 0:64],
        pv[:w, 0:F].rearrange("p (a b) -> p a b", b=64),
        isColT[s][:w, nt, i : i + 1],
    )
    ps = pos_sb[:w, nt, :]
    nc.gpsimd.tensor_copy(
        out=vt[:w, :, 64:70],
        in_=bass.AP(tensor=ps.tensor, offset=ps.offset,
                    ap=[ps.ap[0], [0, 3], ps.ap[-1]]),
    )
    vp[i, s, nt] = vt


def kernel(x1, x2, ln_w, ln_b, qkv_w, proj_w, proj_b):
    wqk, wv, pos_pad, pwt = _host_prep(ln_w, ln_b, qkv_w, proj_w, proj_b)
    xs1 = np.ascontiguousarray(x1.reshape(B, C, N)).astype(ml_dtypes.bfloat16)
    xs2 = np.ascontiguousarray(x2.reshape(B, C, N)).astype(ml_dtypes.bfloat16)
    nc = _build()
    in_maps = []
    for r in range(NCORES):
        in_maps.append(
            {
                "x1s": xs1[r * BP : (r + 1) * BP],
                "x2s": xs2[r * BP : (r + 1) * BP],
                "wqk": wqk,
                "wv": wv,
                "pos": pos_pad,
                "pwt": np.ascontiguousarray(pwt[:, r * OS : (r + 1) * OS]),
                "pb": np.ascontiguousarray(proj_b[None, r * OS : (r + 1) * OS]).astype(np.float32),
            }
        )
    import os

    trace = bool(os.environ.get("BASS_TRACE"))
    res = run_bass_kernel_spmd(nc, in_maps, core_ids=list(range(NCORES)), trace=trace)
    if res.exec_time_ns is not None:
        print(f"HW exec time: {res.exec_time_ns} ns")
    if res.instructions_and_trace:
        print("trace path:", res.instructions_and_trace[1])
    # per-core outputs are in the permuted (half-gather) batch order
    out = np.empty((B, 512), np.float32)
    for r in range(NCORES):
        out[BORDER, r * OS : (r + 1) * OS] = res.results[r]["out"]
    return out


if __name__ == "__main__":
    rng = np.random.default_rng(0)
    ins = {
        "x1": rng.standard_normal((B, C, HG, WG), dtype=np.float32),
        "x2": rng.standard_normal((B, C, HG, WG), dtype=np.float32),
        "ln_w": np.ones(C, np.float32),
        "ln_b": np.zeros(C, np.float32),
        "qkv_w": (rng.standard_normal((3 * F, C)) * C**-0.5).astype(np.float32),
        "proj_w": (rng.standard_normal((512, 6 * 4900)) * (6 * 4900) ** -0.5).astype(np.float32),
        "proj_b": np.zeros(512, np.float32),
    }
    print(kernel(**ins).shape)


# revision 57
# speedup vs baseline: 1.1481x; 1.0771x over previous
"""Trainium2 Bass kernel for nn_EssentialMatixModule.

Dual-softmax cross-attention (LoFTR-style) + bilinear feature + projection.
Data-parallel over batch across 8 cores; proj output-sharded with chunked
AllGathers of the (bf16) feature matrix overlapping the attention phase.

Structure: LN stats (x split over 3 DMA queues), then one fused
software-pipelined loop: per-item QKV streams into the attention blocks
(item-major order, mh-major tail), each block is a 3-stage pipeline
(scores+exp -> colsums+up/us -> f), with E^2 / 1/Zr / vpl emitted
per-row-tile right after each exp so every PE matmul's dependencies are
at least one block old.  Column sums stream through the PE with a
stationary ones vector; 1/sigma is broadcast by DMA (f32 exact).
Feature chunks AllGather in half-batches (permuted batch order, undone
on the host) and the output projection is prefetched and interleaved so
the serial tail is short.  The scalar queue carries only activations
during attention.
"""

import sys

sys.path.insert(0, "/opt/trn_rl_repo")

from contextlib import ExitStack

import ml_dtypes
import numpy as np

import concourse.bass as bass
import concourse.tile as tile
from concourse import bacc, mybir
from concourse.bass_utils import run_bass_kernel_spmd

B, C, HG, WG = 64, 256, 24, 24
N = HG * WG  # 576
H, HD = 3, 64
F = H * HD  # 192
SCALE = HD**-0.5
EPS = 1e-5
NCORES = 8
BP = B // NCORES  # 8 items per core
NT = [128, 128, 128, 128, 64]  # token tiles (sum=576)
# free-dim chunks for N=576 psum; 64-chunk first so each matmul pair ends
# with a 512-col stream that hides the next LDWEIGHTS
NCH = [(512, 64), (0, 512)]
DE = 70  # hd + 6 pos dims
PADMH = 4992  # 39*128, per-(map,head) padded feat block
DIMS = 6 * PADMH  # 29952
OS = 512 // NCORES  # 64 output cols per core
F32 = mybir.dt.float32
BF16 = mybir.dt.bfloat16
AX = mybir.ActivationFunctionType
OP = mybir.AluOpType

# half-batch gather row order: B' = [items 0-3 of each core, items 4-7 of each core]
BORDER = np.array(
    [8 * c + i for i in (0,) for c in range(0)]  # placeholder, built below
)
_rows = []
for c in range(NCORES):
    for i in range(4):
        _rows.append(8 * c + i)
for c in range(NCORES):
    for i in range(4, 8):
        _rows.append(8 * c + i)
BORDER = np.array(_rows)  # BORDER[r] = original batch index of permuted row r


def _host_prep(ln_w, ln_b, qkv_w, proj_w, proj_b):
    ln_w = ln_w.astype(np.float64)
    ln_b = ln_b.astype(np.float64)
    qw = qkv_w.astype(np.float64)
    Wp = qw * ln_w[None, :]  # [576, C]
    r = Wp.sum(axis=1)  # [576]
    t = qw @ ln_b  # [576]

    # per-side packing: side0 tiles hold [k_h; q_h], side1 [q_h; k_h] so the
    # attention matmul operands always share a partition base
    def col(fsl, scale):
        return np.concatenate([Wp[fsl] * scale, (r[fsl] * scale)[:, None],
                               (t[fsl] * scale)[:, None]], axis=1).T

    wqk = np.zeros((2, C + 2, 3 * 128), np.float32)
    for h in range(H):
        qr = slice(h * HD, (h + 1) * HD)
        kr = slice(F + h * HD, F + (h + 1) * HD)
        qcols = col(qr, SCALE)  # [C+2, 64]
        kcols = col(kr, 1.0)
        wqk[0, :, h * 128 : h * 128 + 64] = kcols
        wqk[0, :, h * 128 + 64 : h * 128 + 128] = qcols
        wqk[1, :, h * 128 : h * 128 + 64] = qcols
        wqk[1, :, h * 128 + 64 : h * 128 + 128] = kcols
    wqk = wqk.astype(ml_dtypes.bfloat16)

    wv = np.zeros((C + 2, F), np.float32)
    wv[:C] = Wp[2 * F :].T
    wv[C] = r[2 * F :]
    wv[C + 1] = t[2 * F :]
    wv = wv.astype(ml_dtypes.bfloat16)

    ys = np.linspace(-1.0, 1.0, HG)
    xs = np.linspace(-1.0, 1.0, WG)
    p3 = np.tile(ys, WG)
    p4 = np.repeat(xs, HG)
    pos = np.stack([p3 * p3, p4 * p4, p3 * p4, p3, p4, np.ones_like(p3)], axis=1)
    pos_pad = np.zeros((640, 6), np.float32)
    pos_pad[:N] = pos

    pwt = np.zeros((DIMS, 512), np.float32)
    for mh in range(6):
        blk = proj_w[:, mh * 4900 : (mh + 1) * 4900]  # [512, 4900]
        pwt[mh * PADMH : mh * PADMH + 4900] = blk.T
    pwt = pwt.astype(ml_dtypes.bfloat16)
    return wqk, wv, pos_pad, pwt


def _build():
    nc = bacc.Bacc()
    x1d = nc.declare_dram_parameter("x1s", [BP, C, N], BF16, isOutput=False)
    x2d = nc.declare_dram_parameter("x2s", [BP, C, N], BF16, isOutput=False)
    wqkd = nc.declare_dram_parameter("wqk", [2, C + 2, 3 * 128], BF16, isOutput=False)
    wvd = nc.declare_dram_parameter("wv", [C + 2, F], BF16, isOutput=False)
    posd = nc.declare_dram_parameter("pos", [640, 6], F32, isOutput=False)
    pwtd = nc.declare_dram_parameter("pwt", [DIMS, OS], BF16, isOutput=False)
    pbd = nc.declare_dram_parameter("pb", [1, OS], F32, isOutput=False)
    outd = nc.declare_dram_parameter("out", [B, OS], F32, isOutput=True)
    statsd = nc.dram_tensor("statsd", [2, 2, BP, N], BF16)  # (negmu, sigma)
    isvd = nc.dram_tensor("isvd", [2, BP, N], F32)  # 1/sigma rows, f32
    feat8d = [nc.dram_tensor(f"feat8_{j}", [BP, PADMH], BF16) for j in range(6)]
    # two half-batch gather outputs per mh chunk (items 0-3, items 4-7)
    featAG = [
        [
            nc.dram_tensor(f"featAG_{j}_{hf}", [B // 2, PADMH], BF16, addr_space="Shared")
            for hf in range(2)
        ]
        for j in range(6)
    ]
    xd = [x1d, x2d]

    def bcast_p(sl, p):
        return bass.AP(tensor=sl.tensor, offset=sl.offset, ap=[[0, p]] + list(sl.ap))

    with ExitStack() as ctx:
        tc = ctx.enter_context(tile.TileContext(nc))
        const = ctx.enter_context(tc.tile_pool(name="const", bufs=1))
        xres = ctx.enter_context(tc.tile_pool(name="xres", bufs=1))
        stats = ctx.enter_context(tc.tile_pool(name="stats", bufs=1))
        tmp = ctx.enter_context(tc.tile_pool(name="tmp", bufs=2))
        sb_qk = ctx.enter_context(tc.tile_pool(name="sbqk", bufs=1))
        sb_vp = ctx.enter_context(tc.tile_pool(name="sbvp", bufs=1))
        epool = ctx.enter_context(tc.tile_pool(name="epool", bufs=11))
        e2pool = ctx.enter_context(tc.tile_pool(name="e2pool", bufs=11))
        zpool = ctx.enter_context(tc.tile_pool(name="zpool", bufs=3))
        upool = ctx.enter_context(tc.tile_pool(name="upool", bufs=8))
        vlpool = ctx.enter_context(tc.tile_pool(name="vlpool", bufs=12))
        fpool = ctx.enter_context(tc.tile_pool(name="fpool", bufs=3))
        ftpool = ctx.enter_context(tc.tile_pool(name="ftpool", bufs=3))
        opool = ctx.enter_context(tc.tile_pool(name="opool", bufs=2))
        # PSUM: tag pA [128,576]x2 = 4 banks; pZ [128,576]x1 = 2; pU [128,192]x2 = 2
        psA = ctx.enter_context(tc.tile_pool(name="psA", bufs=2, space="PSUM"))
        psZ = ctx.enter_context(tc.tile_pool(name="psZ", bufs=1, space="PSUM"))
        psU = ctx.enter_context(tc.tile_pool(name="psU", bufs=2, space="PSUM"))

        # ---- constants ----
        wqk_sb = [[], []]
        wv_sb = []
        for k, (k0, kw) in enumerate([(0, 128), (128, 128), (256, 2)]):
            for s in range(2):
                wt = const.tile([kw, 3 * 128], BF16, tag=f"wqk{s}_{k}")
                nc.sync.dma_start(out=wt, in_=wqkd[s, k0 : k0 + kw, :])
                wqk_sb[s].append(wt)
            vt = const.tile([kw, F], BF16, tag=f"wv{k}")
            nc.sync.dma_start(out=vt, in_=wvd[k0 : k0 + kw, :])
            wv_sb.append(vt)
        pos_sb = const.tile([128, 5, 6], F32, tag="pos")
        nc.sync.dma_start(out=pos_sb, in_=posd.rearrange("(t p) e -> p t e", p=128))
        ind8_sb = const.tile([128, BP, BP], BF16, tag="ind8")
        nc.vector.memset(ind8_sb, 0.0)
        for i in range(BP):
            nc.vector.memset(ind8_sb[:, i, i : i + 1], 1.0)
        onesb_sb = const.tile([128, 32], BF16, tag="onesb")
        nc.vector.memset(onesb_sb, 1.0)
        epssb = const.tile([32, 1], F32, tag="eps")
        nc.vector.memset(epssb, EPS)
        pb_sb = const.tile([B, OS], F32, tag="pb")
        nc.gpsimd.dma_start(out=pb_sb, in_=bcast_p(pbd[0, :], B))
        # HAM keep-alive source: dependency-free matmul fodder. The PE clock
        # gate only unthrottles after a fully-busy 4096-cycle window; these
        # extend each block's gapless stream past that and absorb the idle
        # that would otherwise re-throttle it.
        dsrc = const.tile([128, 512], BF16, tag="dsrc")
        nc.vector.memset(dsrc, 1.0)
        zpad = const.tile([1, 552], BF16, tag="zpad")
        nc.vector.memset(zpad, 0.0)
        for j in range(6):
            for ib in range(BP):
                nc.gpsimd.dma_start(
                    out=feat8d[j][ib, 4900:PADMH],
                    in_=bass.AP(tensor=zpad.tensor, offset=zpad.offset, ap=[[1, 1], [1, 92]]),
                )

        # ---- phase 1a: LN stats (x split across 4 DMA queues) ----
        qdma = [nc.sync, nc.gpsimd, nc.scalar]
        qrot = [0]

        def load_x(xt, s, i, k):
            # split each [128, N] tile into partition halves spread over the
            # DMA queues so startup isn't bound by one DMA engine's bandwidth
            for hh in range(2):
                q = qdma[qrot[0] % 3]
                qrot[0] += 1
                q.dma_start(
                    out=xt[hh * 64 : (hh + 1) * 64],
                    in_=xd[s][i, k * 128 + hh * 64 : k * 128 + (hh + 1) * 64, :],
                )

        isColT = []
        for s in range(2):
            psum_s = psA.tile([128, N], F32, tag="pA")
            psum_q = psA.tile([128, N], F32, tag="pA")
            for i in range(BP):
                for k in range(2):
                    xt = xres.tile([128, N], BF16, tag="x", bufs=6, name="xt")
                    load_x(xt, s, i, k)
                    xq = tmp.tile([128, N], BF16, tag="xsq")
                    nc.vector.tensor_mul(xq, xt, xt)
                    st = i == 0 and k == 0
                    for c0, cw in NCH:
                        nc.tensor.matmul(
                            psum_s[:BP, c0 : c0 + cw], ind8_sb[:, i, :], xt[:, c0 : c0 + cw],
                            start=st, stop=(i == BP - 1 and k == 1),
                        )
                        nc.tensor.matmul(
                            psum_q[:BP, c0 : c0 + cw], ind8_sb[:, i, :], xq[:, c0 : c0 + cw],
                            start=st, stop=(i == BP - 1 and k == 1),
                        )
            mean = stats.tile([32, N], F32, tag="mean")
            ex2 = stats.tile([32, N], F32, tag="ex2")
            nc.vector.tensor_scalar_mul(mean[:BP], psum_s[:BP], 1.0 / C)
            nc.vector.tensor_scalar_mul(ex2[:BP], psum_q[:BP], 1.0 / C)
            var = stats.tile([32, N], F32, tag="var")
            nc.vector.scalar_tensor_tensor(
                out=var[:BP], in0=mean[:BP], scalar=-1.0, in1=mean[:BP], op0=OP.mult, op1=OP.mult
            )
            nc.vector.tensor_add(var[:BP], var[:BP], ex2[:BP])
            sig = stats.tile([32, N], F32, tag="sig")
            nc.scalar.activation(out=sig[:BP], in_=var[:BP], func=AX.Sqrt, bias=epssb[:BP])
            isvf = stats.tile([32, N], F32, tag=f"isvf{s}")
            nc.vector.reciprocal(isvf[:BP], sig[:BP])
            negmu = stats.tile([32, N], BF16, tag="negmu")
            nc.vector.tensor_scalar_mul(negmu[:BP], mean[:BP], -1.0)
            sigb = stats.tile([32, N], BF16, tag="sigb")
            nc.vector.tensor_copy(sigb[:BP], sig[:BP])
            # stats round-trips go on the scalar queue (idle until attention)
            nc.scalar.dma_start(out=statsd[s, 0], in_=negmu[:BP])
            nc.scalar.dma_start(out=statsd[s, 1], in_=sigb[:BP])
            nc.scalar.dma_start(out=isvd[s], in_=isvf[:BP])
            zt_is = stats.tile([32, 18, 32], F32, tag="zt_is")
            nc.vector.transpose(out=zt_is, in_=isvf.rearrange("p (g q) -> p g q", q=32))
            ict = const.tile([128, 5, BP], F32, tag=f"iscol{s}")
            for a in range(4):
                ng = 5 if a < 2 else 4
                nc.vector.tensor_copy(
                    out=ict[32 * a : 32 * a + 32, 0:ng, :],
                    in_=zt_is[:, a : 18 : 4, 0:BP],
                )
            isColT.append(ict)

        # ---- phase 1b: QKV emitters (executed fused into the block loop) ----
        qs = {}
        ks = {}
        vp = {}

        def fetch_qkv(i, s):
            xe = stats.tile([2, N], BF16, tag="xe", bufs=6, name="xe")
            nc.scalar.dma_start(out=xe, in_=statsd[s, :, i, :])
            isb = tmp.tile([128, N], F32, tag="isb", bufs=6, name="isb")
            nc.scalar.dma_start(out=isb, in_=bcast_p(isvd[s, i, :], 128))
            xt0 = xres.tile([128, N], BF16, tag="x2", bufs=8, name="xt0")
            load_x(xt0, s, i, 0)
            xt1 = xres.tile([128, N], BF16, tag="x2", bufs=8, name="xt1")
            load_x(xt1, s, i, 1)
            return xe, isb, xt0, xt1

        def emit_qkv(i, s, fetched_in):
            xe, isb, xt0, xt1 = fetched_in
            rhs3 = [xt0, xt1, xe]
            pqs = []
            for h in range(H):
                pq = psA.tile([128, N], F32, tag="pA")
                for k in range(3):
                    for c0, cw in NCH:
                        nc.tensor.matmul(
                            pq[:, c0 : c0 + cw],
                            wqk_sb[s][k][:, h * 128 : (h + 1) * 128],
                            rhs3[k][:, c0 : c0 + cw],
                            start=(k == 0), stop=(k == 2),
                        )
                pqs.append(pq)
                if h >= 1:  # consume pq(h-1) so the pA pool never blocks
                    _qk_emit(nc, sb_qk, qs, ks, pqs[h - 1], isb, i, s, h - 1)
            pvs = []
            for nt in range(5):
                w = NT[nt]
                n0 = nt * 128
                pv = psA.tile([128, F], F32, tag="pA")
                for k in range(3):
                    nc.tensor.matmul(
                        pv[:w],
                        rhs3[k][:, n0 : n0 + w],
                        wv_sb[k],
                        start=(k == 0), stop=(k == 2),
                    )
                pvs.append(pv)
                if nt == 0:
                    _qk_emit(nc, sb_qk, qs, ks, pqs[2], isb, i, s, 2)
                if nt >= 2:
                    _vt_emit(nc, sb_vp, vp, pos_sb, isColT, pvs[nt - 2], i, s, nt - 2)
            _vt_emit(nc, sb_vp, vp, pos_sb, isColT, pvs[3], i, s, 3)
            _vt_emit(nc, sb_vp, vp, pos_sb, isColT, pvs[4], i, s, 4)

        # ---- phase 2: pipelined attention; half-gathers + proj overlap ----
        oacc = opool.tile([B, OS], F32, tag="oacc")
        nc.vector.memset(oacc, 0.0)

        GSZ = 13
        pw5 = []  # persistent proj weights for the split mh=5 projection

        def fetch_proj(mh, half=None):
            # issue the DMAs (feature transposes + weights) well before the
            # matmuls so the PE never head-blocks on them
            fts = []
            pws = []
            for gi, g0 in enumerate(range(0, 39, GSZ)):
                gsl = slice(g0 * 128, (g0 + GSZ) * 128)
                psl = slice(mh * PADMH + g0 * 128, mh * PADMH + (g0 + GSZ) * 128)
                if half is None:
                    ft = ftpool.tile([128, GSZ, B], BF16, tag="ft", bufs=6)
                    # two half-batch transposes into disjoint column halves;
                    # the column order is the permuted batch order BORDER
                    nc.sync.dma_start_transpose(
                        out=ft[:, :, 0 : B // 2], in_=featAG[mh][0][:, gsl]
                    )
                    nc.sync.dma_start_transpose(
                        out=ft[:, :, B // 2 : B], in_=featAG[mh][1][:, gsl]
                    )
                else:
                    ft = ftpool.tile([128, GSZ, B // 2], BF16, tag="fth")
                    nc.sync.dma_start_transpose(
                        out=ft, in_=featAG[mh][half][:, gsl]
                    )
                fts.append(ft)
                if half == 1:
                    pws.append(pw5[gi])
                else:
                    pw = ftpool.tile(
                        [128, GSZ, OS], BF16,
                        tag=("pw" if half is None else f"pw5_{gi}"),
                        bufs=(6 if half is None else 1),
                    )
                    nc.gpsimd.dma_start(
                        out=pw, in_=pwtd[psl].rearrange("(j p) o -> p j o", p=128)
                    )
                    if half == 0:
                        pw5.append(pw)
                    pws.append(pw)
            return fts, pws

        def emit_proj_mm(fetched, half=None):
            fts, pws = fetched
            for gi in range(3):
                opsum = psA.tile([64, OS], F32, tag="pA")
                for j in range(GSZ):
                    nc.tensor.matmul(
                        opsum[0 : 64 if half is None else 32],
                        fts[gi][:, j, :], pws[gi][:, j, :],
                        start=(j == 0), stop=(j == GSZ - 1),
                    )
                if half is None:
                    nc.vector.tensor_add(oacc, oacc, opsum)
                else:
                    hsl = slice(half * 32, half * 32 + 32)
                    nc.vector.tensor_add(oacc[hsl], oacc[hsl], opsum[0:32])

        def emit_gather(j, hf):
            nc.gpsimd.collective_compute(
                "AllGather",
                OP.bypass,
                replica_groups=[list(range(NCORES))],
                ins=[feat8d[j][hf * 4 : hf * 4 + 4, :]],
                outs=[featAG[j][hf][:]],
            )

        # fused block order: item-major for items 0-5 (QKV streams into the
        # pipeline), mh-major tail for items 6-7 (spreads the final gathers)
        blocks = [(m, h, i) for i in range(6) for m in range(2) for h in range(H)]
        blocks += [(m, h, i) for m in range(2) for h in range(H) for i in (6, 7)]
        fetched = {}

        class Blk:
            __slots__ = ("m", "h", "i", "et", "e2", "zr5", "rzr5", "zcp", "rzc",
                         "vpl", "us", "fps")

        def emit_pa_tile(b, nt):
            m, h, i = b.m, b.h, b.i
            qside = 1 - m
            w = NT[nt]
            n0 = nt * 128
            pa = psA.tile([128, N], F32, tag="pA")
            for c0, cw in NCH:
                nc.tensor.matmul(
                    pa[:w, c0 : c0 + cw],
                    qs[i, qside, h][:, n0 : n0 + w],
                    ks[i, m, h][:, c0 : c0 + cw],
                    start=True, stop=True,
                )
            et = epool.tile([128, N], BF16, tag="E")
            nc.scalar.activation(
                out=et[:w], in_=pa[:w], func=AX.Exp,
                accum_out=b.zr5[:w, nt : nt + 1],
            )
            b.et.append(et)
            # per-tile consumers: E^2, 1/Zr for this row-tile, and the
            # 1/Zr-scaled vp copy -- all ready well before up(b) next step
            _e2_emit(nc, e2pool, b, nt)
            nc.vector.reciprocal(b.rzr5[:w, nt : nt + 1], b.zr5[:w, nt : nt + 1])
            vpl = vlpool.tile([128, 72], BF16, tag="vpl")
            nc.vector.tensor_scalar_mul(
                vpl[:w, 0:70], vp[b.i, b.m, nt][:w, b.h, 0:70],
                b.rzr5[:w, nt : nt + 1],
            )
            b.vpl.append(vpl)

        def emit_zc(p):
            # streaming column sums: ones stationary, E moving (solid streams)
            p.zcp = psZ.tile([32, N], F32, tag="pZ")
            for nt in range(5):
                w = NT[nt]
                for c0, cw in NCH:
                    nc.tensor.matmul(
                        p.zcp[:, c0 : c0 + cw], onesb_sb[:w, :], p.et[nt][:w, c0 : c0 + cw],
                        start=(nt == 0), stop=(nt == 4),
                    )

        def emit_zcpost(p):
            # transpose/extract 1/Zc into per-partition layout [128, 5]
            zt = tmp.tile([32, 18, 32], F32, tag="zt")
            nc.vector.transpose(out=zt, in_=p.zcp.rearrange("p (g q) -> p g q", q=32))
            rz32 = zpool.tile([32, 18], F32, tag="rz32")
            nc.vector.reciprocal(rz32, zt[:, :, 0])
            p.rzc = zpool.tile([128, 8], F32, tag="rzc")
            for a in range(4):
                ng = 5 if a < 2 else 4
                nc.gpsimd.tensor_copy(
                    out=p.rzc[32 * a : 32 * a + 32, 0:ng],
                    in_=rz32[:, a : 18 : 4],
                )

        def make_up_us(p):
            ups = []

            def one_up(mc):
                w2 = NT[mc]
                up = psU.tile([128, 72], F32, tag="pU")
                for nt in range(5):
                    w = NT[nt]
                    nc.tensor.matmul(
                        up[:w2, 0:70],
                        p.e2[nt][:w, mc * 128 : mc * 128 + w2],
                        p.vpl[nt][:w, 0:70],
                        start=(nt == 0), stop=(nt == 4),
                    )
                ups.append(up)

            def one_us(mc):
                w2 = NT[mc]
                us = upool.tile([128, 72], BF16, tag="us", bufs=12)
                nc.vector.tensor_scalar_mul(
                    us[:w2, 0:70], ups[mc][:w2, 0:70], p.rzc[:w2, mc : mc + 1]
                )
                p.us.append(us)

            return one_up, one_us

        def emit_f(p):
            # f-chain runs one step after up/us: its deps are a step old, so
            # the PE never micro-waits on the us copies
            p.fps = psU.tile([128, 72], F32, tag="pU")
            for mc in range(5):
                w2 = NT[mc]
                nc.tensor.matmul(
                    p.fps[0:70, 0:70],
                    p.us[mc][:w2, 0:70],
                    vp[p.i, p.m, mc][:w2, p.h, 0:70],
                    start=(mc == 0), stop=(mc == 4),
                )

        def emit_fstore(p):
            mh = p.m * 3 + p.h
            fb = fpool.tile([70, 70], BF16, tag="fb")
            nc.vector.tensor_copy(out=fb, in_=p.fps[0:70, 0:70])
            nc.sync.dma_start(
                out=feat8d[mh][p.i, 0:4900].rearrange("(d e) -> d e", e=70),
                in_=fb,
            )
            if p.i == 3:
                emit_gather(mh, 0)
            elif p.i == BP - 1:
                emit_gather(mh, 1)

        # prologue QKV: items 0 and 1 fully before the first block
        qkv_pref = {}
        for ii in range(2):
            for s in range(2):
                qkv_pref[ii, s] = fetch_qkv(ii, s)
        for ii in range(2):
            for s in range(2):
                emit_qkv(ii, s, qkv_pref.pop((ii, s)))

        # per-step hooks in the mh-major tail: after tail-step t, proj work
        tail_fetch = {5: (5, 0), 6: (0, None), 8: (1, None), 10: (2, None)}
        tail_mm = {7: (5, 0), 8: (0, None), 10: (1, None)}

        p1 = None  # previous block (zc/up/us stage)
        p2 = None  # block before that (f stage)
        for bi, (m, h, i) in enumerate(blocks):
            tail_t = bi - 36  # >=0 inside the mh-major tail section
            b = Blk()
            b.m, b.h, b.i = m, h, i
            b.zr5 = zpool.tile([128, 8], F32, tag="zr5")
            b.rzr5 = zpool.tile([128, 8], F32, tag="rzr5")
            b.et = []
            b.e2 = []
            b.vpl = []
            b.us = []
            emit_pa_tile(b, 0)
            emit_pa_tile(b, 1)
            if p1 is not None:
                emit_zc(p1)  # PE: solid 576-col streams
            # dependency-free keep-alive matmuls: keep the gapless stream
            # long enough to cover a full HAM window and fill warm-state idle
            dmy = psU.tile([128, 512], F32, tag="pU", name="dmy")
            for _ in range(7):
                nc.tensor.matmul(dmy[0:32, :], onesb_sb, dsrc, start=True, stop=True)
            dscr = tmp.tile([1, 8], F32, tag="dscr", bufs=2, name="dscr")
            nc.vector.tensor_copy(dscr[0:1, 0:1], dmy[0:1, 0:1])
            emit_pa_tile(b, 2)
            if p2 is not None:
                emit_f(p2)  # PE shorts; all deps one step old
            if p1 is not None:
                one_up, one_us = make_up_us(p1)
                one_up(0)
                one_up(1)
            emit_pa_tile(b, 3)
            if p1 is not None:
                # zcpost late on DVE so it never head-blocks the e2/vpl
                # stream that next step's up matmuls depend on
                emit_zcpost(p1)
                one_us(0)
                one_us(1)
                one_up(2)
                one_us(2)
            emit_pa_tile(b, 4)
            if p1 is not None:
                one_up(3)
                one_us(3)
                one_up(4)
                one_us(4)
            if p2 is not None:
                emit_fstore(p2)  # DVE copy + sync DMA + gather hooks
            # streamed-in QKV for item i+2 (item-major section only)
            if i < 6 and bi < 36:
                step6 = bi % 6
                if i + 2 < 8:
                    if step6 == 0:
                        qkv_pref[i + 2, 0] = fetch_qkv(i + 2, 0)
                    elif step6 == 1:
                        emit_qkv(i + 2, 0, qkv_pref.pop((i + 2, 0)))
                    elif step6 == 2:
                        qkv_pref[i + 2, 1] = fetch_qkv(i + 2, 1)
                    elif step6 == 3:
                        emit_qkv(i + 2, 1, qkv_pref.pop((i + 2, 1)))
            if tail_t >= 0:
                if tail_t in tail_fetch:
                    j, hf = tail_fetch[tail_t]
                    fetched[j, hf] = fetch_proj(j, half=hf)
                if tail_t in tail_mm:
                    j, hf = tail_mm[tail_t]
                    emit_proj_mm(fetched.pop((j, hf)), half=hf)
            p2 = p1
            p1 = b

        # epilogue: drain the two pending blocks, then the remaining projs
        emit_f(p2)
        emit_zc(p1)
        emit_zcpost(p1)
        emit_fstore(p2)
        one_up, one_us = make_up_us(p1)
        for mc in range(5):
            one_up(mc)
            one_us(mc)
        emit_proj_mm(fetched.pop((2, None)))
        fetched[3] = fetch_proj(3)
        emit_f(p1)
        emit_fstore(p1)  # fires gather(5, 1)
        emit_proj_mm(fetched.pop(3))
        fetched[4] = fetch_proj(4)
        emit_proj_mm(fetched.pop(4))
        emit_proj_mm(fetch_proj(5, half=1), half=1)
        osb = opool.tile([B, OS], F32, tag="osb")
        nc.vector.tensor_add(osb, oacc, pb_sb)
        nc.vector.tensor_scalar_max(osb, osb, 0.0)
        nc.sync.dma_start(out=outd[:], in_=osb)

    nc.compile()
    return nc


def _e2_emit(nc, e2pool, b, nt):
    w = NT[nt]
    e2 = e2pool.tile([128, N], BF16, tag="E2", name="e2")
    nc.vector.tensor_mul(e2[:w], b.et[nt][:w], b.et[nt][:w])
    b.e2.append(e2)


def _qk_emit(nc, sb_qk, qs, ks, pq, isb, i, s, h):
    # rotating ring: item i's tiles die once its 6 blocks are done
    qk = sb_qk.tile([128, N], BF16, tag=f"qk{s}_{h}", bufs=4, name="qk")
    nc.vector.tensor_mul(qk, pq, isb)
    if s == 0:
        ks[i, s, h] = qk[0:64, :]
        qs[i, s, h] = qk[64:128, :]
    else:
        qs[i, s, h] = qk[0:64, :]
        ks[i, s, h] = qk[64:128, :]


def _vt_emit(nc, sb_vp, vp, pos_sb, isColT, pv, i, s, nt):
    w = NT[nt]
    vt = sb_vp.tile([128, 3, 72], mybir.dt.bfloat16, tag=f"vp{s}_{nt}", bufs=4, name="vt")
    nc.vector.tensor_scalar_mul(
        vt[:w, :, 0:64],
        pv[:w, 0:F].rearrange("p (a b) -> p a b", b=64),
        isColT[s][:w, nt, i : i + 1],
    )
    ps = pos_sb[:w, nt, :]
    nc.gpsimd.tensor_copy(
        out=vt[:w, :, 64:70],
        in_=bass.AP(tensor=ps.tensor, offset=ps.offset,
                    ap=[ps.ap[0], [0, 3], ps.ap[-1]]),
    )
    vp[i, s, nt] = vt


def kernel(x1, x2, ln_w, ln_b, qkv_w, proj_w, proj_b):
    wqk, wv, pos_pad, pwt = _host_prep(ln_w, ln_b, qkv_w, proj_w, proj_b)
    xs1 = np.ascontiguousarray(x1.reshape(B, C, N)).astype(ml_dtypes.bfloat16)
    xs2 = np.ascontiguousarray(x2.reshape(B, C, N)).astype(ml_dtypes.bfloat16)
    nc = _build()
    in_maps = []
    for r in range(NCORES):
        in_maps.append(
            {
                "x1s": xs1[r * BP : (r + 1) * BP],
                "x2s": xs2[r * BP : (r + 1) * BP],
                "wqk": wqk,
                "wv": wv,
                "pos": pos_pad,
                "pwt": np.ascontiguousarray(pwt[:, r * OS : (r + 1) * OS]),
                "pb": np.ascontiguousarray(proj_b[None, r * OS : (r + 1) * OS]).astype(np.float32),
            }
        )
    import os

    trace = bool(os.environ.get("BASS_TRACE"))
    res = run_bass_kernel_spmd(nc, in_maps, core_ids=list(range(NCORES)), trace=trace)
    if res.exec_time_ns is not None:
        print(f"HW exec time: {res.exec_time_ns} ns")
    if res.instructions_and_trace:
        print("trace path:", res.instructions_and_trace[1])
    # per-core outputs are in the permuted (half-gather) batch order
    out = np.empty((B, 512), np.float32)
    for r in range(NCORES):
        out[BORDER, r * OS : (r + 1) * OS] = res.results[r]["out"]
    return out


if __name__ == "__main__":
    rng = np.random.default_rng(0)
    ins = {
        "x1": rng.standard_normal((B, C, HG, WG), dtype=np.float32),
        "x2": rng.standard_normal((B, C, HG, WG), dtype=np.float32),
        "ln_w": np.ones(C, np.float32),
        "ln_b": np.zeros(C, np.float32),
        "qkv_w": (rng.standard_normal((3 * F, C)) * C**-0.5).astype(np.float32),
        "proj_w": (rng.standard_normal((512, 6 * 4900)) * (6 * 4900) ** -0.5).astype(np.float32),
        "proj_b": np.zeros(512, np.float32),
    }
    print(kernel(**ins).shape)
